# revision 1
# baseline (speedup 1.0000x reference)
"""Trainium2 Bass kernel for the DISL loss (nn_DISL_Loss).

Strategy (data-parallel over batch, 8 cores):
  Pass 1 (device): per-core contraction over its 2048 (b,t) rows:
    G_A = O_A^T V, G_F = O_F^T V  [512,1024] partials, column sums of
    squares of V/O_A/O_F (for sim normalization), and the triplet
    weighted row-sum T = W^T vaf_satt [3,1024] partials.
  Host: all-reduce partials, normalize -> sim, greedy unique assignment
    (tiny, sequential), build one-hot permutation matrices P_A, P_F.
  Pass 2 (device): per-core gathered-column dots via one-hot matmul:
    Ap = O_A @ P_A, Fp = O_F @ P_F; per-row n1=<V,Ap>, n2=<V,Fp>,
    n3=<Ap,Fp> and row sums of squares of V/O_A/O_F.
  Host: cos/CE/BCE/triplet final combine (small tensors only).

Key identity: ext is a permutation of 0..1023, so ||pA_row|| equals
||O_A_row|| and no gathered norms are needed.
"""

import numpy as np
import ml_dtypes

B, T, M, OM = 64, 256, 1024, 512
N_CORES = 8
SPC = B // N_CORES          # samples per core
RPC = SPC * T               # rows per core
P = 128

_prog_cache = {}


# ---------------------------------------------------------------- pass 1
def _build_pass1(rows, g_f32r=False):
    from concourse import bacc, mybir
    from concourse.tile import TileContext

    f32 = mybir.dt.float32
    gdt = mybir.dt.float32r if g_f32r else f32
    kt = rows // P
    ST = 4 if kt % 4 == 0 else (2 if kt % 2 == 0 else 1)
    NS = kt // ST
    nc = bacc.Bacc()
    v_d = nc.declare_dram_parameter("v", [rows, M], gdt, isOutput=False)
    oa_d = nc.declare_dram_parameter("oa", [rows, OM], gdt, isOutput=False)
    of_d = nc.declare_dram_parameter("of", [rows, OM], gdt, isOutput=False)
    vaf_d = nc.declare_dram_parameter("vaf", [rows, M], f32, isOutput=False)
    w_d = nc.declare_dram_parameter("w", [rows, 4], f32, isOutput=False)
    ga_d = nc.declare_dram_parameter("ga", [OM, M], f32, isOutput=True)
    gf_d = nc.declare_dram_parameter("gf", [OM, M], f32, isOutput=True)
    csq_d = nc.declare_dram_parameter("csq", [3, M], f32, isOutput=True)
    tm_d = nc.declare_dram_parameter("tm", [4, M], f32, isOutput=True)

    def sup(dram, s):  # super-tile view: ST row-tiles in one DMA
        return dram[s * ST * P:(s + 1) * ST * P, :].rearrange(
            "(t p) m -> p t m", p=P)

    with TileContext(nc) as tc:
        with (
            tc.tile_pool(name="vres", bufs=NS) as vpool,
            tc.tile_pool(name="stream", bufs=2) as spool,
            tc.tile_pool(name="sq", bufs=2) as qpool,
            tc.tile_pool(name="acc", bufs=1) as apool,
            tc.tile_pool(name="cb", bufs=3) as cbpool,
            tc.tile_pool(name="const", bufs=1) as cpool,
        ):
            ones = cpool.tile([P, 1], f32, tag="ones")
            nc.vector.memset(ones[:], 1.0)
            accv = apool.tile([P, ST, M], f32, tag="accv")
            acca = apool.tile([P, ST, OM], f32, tag="acca")
            accf = apool.tile([P, ST, OM], f32, tag="accf")

            vtiles = []
            for s in range(NS):
                vt = vpool.tile([P, ST, M], gdt, tag="v", name=f"vres{s}")
                eng = nc.sync if s % 2 == 0 else nc.scalar
                eng.dma_start(vt[:], sup(v_d, s))
                vtiles.append(vt)

            # ---- phase A: G_A accumulation + squares of v, oa ----
            with tc.tile_pool(name="psA", bufs=8, space="PSUM") as ppa:
                gps = [ppa.tile([P, 512], f32, tag="g", name=f"gpsA{i}")
                       for i in range(8)]
                for s in range(NS):
                    oat = spool.tile([P, ST, OM], gdt, tag="oaf")
                    eng = nc.sync if s % 2 == 0 else nc.scalar
                    eng.dma_start(oat[:], sup(oa_d, s))
                    for j in range(ST):
                        for mc in range(4):
                            for nh in range(2):
                                nc.tensor.matmul(
                                    gps[mc * 2 + nh][:],
                                    lhsT=oat[:, j, mc * P:(mc + 1) * P],
                                    rhs=vtiles[s][:, j, nh * 512:(nh + 1) * 512],
                                    start=(s == 0 and j == 0),
                                    stop=(s == NS - 1 and j == ST - 1),
                                )
                    for j in range(ST):
                        if s == 0:
                            nc.vector.tensor_mul(accv[:, j, :],
                                                 vtiles[s][:, j, :].bitcast(f32),
                                                 vtiles[s][:, j, :].bitcast(f32))
                            nc.vector.tensor_mul(acca[:, j, :], oat[:, j, :].bitcast(f32),
                                                 oat[:, j, :].bitcast(f32))
                        else:
                            sqv = qpool.tile([P, M], f32, tag="sqv",
                                             name=f"sqv{s}_{j}")
                            nc.vector.tensor_mul(sqv[:], vtiles[s][:, j, :].bitcast(f32),
                                                 vtiles[s][:, j, :].bitcast(f32))
                            nc.vector.tensor_add(accv[:, j, :], accv[:, j, :],
                                                 sqv[:])
                            sqa = qpool.tile([P, OM], f32, tag="sqa",
                                             name=f"sqa{s}_{j}")
                            nc.vector.tensor_mul(sqa[:], oat[:, j, :].bitcast(f32),
                                                 oat[:, j, :].bitcast(f32))
                            nc.vector.tensor_add(acca[:, j, :], acca[:, j, :],
                                                 sqa[:])
                for mc in range(4):
                    cb = cbpool.tile([P, M], f32, tag="cb", name=f"cba{mc}")
                    nc.scalar.copy(cb[:, 0:512], gps[mc * 2][:])
                    nc.scalar.copy(cb[:, 512:M], gps[mc * 2 + 1][:])
                    eng = nc.sync if mc % 2 == 0 else nc.scalar
                    eng.dma_start(ga_d[mc * P:(mc + 1) * P, :], cb[:])

            # ---- phase B: G_F accumulation + squares of of ----
            with tc.tile_pool(name="psB", bufs=8, space="PSUM") as ppb:
                gps = [ppb.tile([P, 512], f32, tag="g", name=f"gpsB{i}")
                       for i in range(8)]
                for s in range(NS):
                    oft = spool.tile([P, ST, OM], gdt, tag="oaf", name=f"ofst{s}")
                    eng = nc.sync if s % 2 == 0 else nc.scalar
                    eng.dma_start(oft[:], sup(of_d, s))
                    for j in range(ST):
                        for mc in range(4):
                            for nh in range(2):
                                nc.tensor.matmul(
                                    gps[mc * 2 + nh][:],
                                    lhsT=oft[:, j, mc * P:(mc + 1) * P],
                                    rhs=vtiles[s][:, j, nh * 512:(nh + 1) * 512],
                                    start=(s == 0 and j == 0),
                                    stop=(s == NS - 1 and j == ST - 1),
                                )
                    for j in range(ST):
                        if s == 0:
                            nc.vector.tensor_mul(accf[:, j, :], oft[:, j, :].bitcast(f32),
                                                 oft[:, j, :].bitcast(f32))
                        else:
                            sqf = qpool.tile([P, OM], f32, tag="sqa",
                                             name=f"sqf{s}_{j}")
                            nc.vector.tensor_mul(sqf[:], oft[:, j, :].bitcast(f32),
                                                 oft[:, j, :].bitcast(f32))
                            nc.vector.tensor_add(accf[:, j, :], accf[:, j, :],
                                                 sqf[:])
                for mc in range(4):
                    cb = cbpool.tile([P, M], f32, tag="cb", name=f"cbf{mc}")
                    nc.scalar.copy(cb[:, 0:512], gps[mc * 2][:])
                    nc.scalar.copy(cb[:, 512:M], gps[mc * 2 + 1][:])
                    eng = nc.sync if mc % 2 == 0 else nc.scalar
                    eng.dma_start(gf_d[mc * P:(mc + 1) * P, :], cb[:])

            # ---- phase C: triplet weighted row-sum + csq reductions ----
            with tc.tile_pool(name="psC", bufs=1, space="PSUM") as ppc:
                wt_all = cpool.tile([P, kt, 4], f32, tag="w_all")
                nc.gpsimd.dma_start(
                    wt_all[:], w_d.rearrange("(k p) c -> p k c", p=P))
                tmps = ppc.tile([4, M], f32, tag="tm")
                for s in range(NS):
                    vft = spool.tile([P, ST, M], f32, tag="vaf")
                    eng = nc.sync if s % 2 == 0 else nc.scalar
                    eng.dma_start(vft[:], sup(vaf_d, s))
                    for j in range(ST):
                        for nh in range(2):
                            nc.tensor.matmul(
                                tmps[:, nh * 512:(nh + 1) * 512],
                                lhsT=wt_all[:, s * ST + j, :],
                                rhs=vft[:, j, nh * 512:(nh + 1) * 512],
                                start=(s == 0 and j == 0),
                                stop=(s == NS - 1 and j == ST - 1),
                            )
                cb = cbpool.tile([4, M], f32, tag="cbt")
                nc.scalar.copy(cb[:], tmps[:])
                nc.sync.dma_start(tm_d[:, :], cb[:])

                csqps = ppc.tile([1, M], f32, tag="csqv")
                for nh in range(2):
                    for t in range(ST):
                        nc.tensor.matmul(
                            csqps[:, nh * 512:(nh + 1) * 512],
                            lhsT=ones[:],
                            rhs=accv[:, t, nh * 512:(nh + 1) * 512],
                            start=(t == 0), stop=(t == ST - 1),
                        )
                cbv = cbpool.tile([1, M], f32, tag="cbv")
                nc.scalar.copy(cbv[:], csqps[:])
                nc.sync.dma_start(csq_d[0:1, :], cbv[:])

                csqps2 = ppc.tile([1, M], f32, tag="csqa")
                for t in range(ST):
                    nc.tensor.matmul(csqps2[:, 0:512], lhsT=ones[:],
                                     rhs=acca[:, t, :],
                                     start=(t == 0), stop=(t == ST - 1))
                    nc.tensor.matmul(csqps2[:, 512:M], lhsT=ones[:],
                                     rhs=accf[:, t, :],
                                     start=(t == 0), stop=(t == ST - 1))
                cba = cbpool.tile([1, M], f32, tag="cbv")
                nc.scalar.copy(cba[:], csqps2[:])
                nc.scalar.dma_start(csq_d[1:2, :], cba[:])
    nc.finalize()
    return nc


# ---------------------------------------------------------------- pass 2
def _build_pass2(rows):
    from concourse import bacc, mybir
    from concourse.tile import TileContext
    from concourse.masks import make_identity

    f32 = mybir.dt.float32
    bf16 = mybir.dt.bfloat16
    kt = rows // P
    ST = 4 if kt % 4 == 0 else (2 if kt % 2 == 0 else 1)
    NS = kt // ST
    nc = bacc.Bacc()
    v_d = nc.declare_dram_parameter("v", [rows, M], f32, isOutput=False)
    oa_d = nc.declare_dram_parameter("oa", [rows, OM], f32, isOutput=False)
    of_d = nc.declare_dram_parameter("of", [rows, OM], f32, isOutput=False)
    pa_d = nc.declare_dram_parameter("pa", [OM, M], bf16, isOutput=False)
    pf_d = nc.declare_dram_parameter("pf", [OM, M], bf16, isOutput=False)
    st_d = nc.declare_dram_parameter("st", [rows, 8], f32, isOutput=True)

    def sup(dram, s):
        return dram[s * ST * P:(s + 1) * ST * P, :].rearrange(
            "(t p) m -> p t m", p=P)

    with TileContext(nc) as tc:
        with (
            tc.tile_pool(name="const", bufs=1) as cpool,
            tc.tile_pool(name="ores", bufs=2 * NS) as opool,
            tc.tile_pool(name="obts", bufs=2 * kt) as tpool,
            tc.tile_pool(name="stream", bufs=2) as spool,
            tc.tile_pool(name="gsbp", bufs=4) as gpool,
            tc.tile_pool(name="scr", bufs=2) as qpool,
        ):
            X = mybir.AxisListType.X
            ident = cpool.tile([P, P], f32, tag="ident")
            make_identity(nc, ident[:])
            pa_sb = cpool.tile([P, 4, M], bf16, tag="pa")
            pf_sb = cpool.tile([P, 4, M], bf16, tag="pf")
            for i in range(4):
                eng = nc.sync if i % 2 == 0 else nc.scalar
                eng.dma_start(pa_sb[:, i, :], pa_d[i * P:(i + 1) * P, :])
                eng.dma_start(pf_sb[:, i, :], pf_d[i * P:(i + 1) * P, :])
            stat_all = cpool.tile([P, kt, 8], f32, tag="stat_all")
            nc.vector.memset(stat_all[:], 0.0)

            oa_res, of_res, obT = [], [], {}
            # ---- phase T: load O tensors, transpose all row-tiles ----
            with tc.tile_pool(name="ptr", bufs=6, space="PSUM") as ptr:
                for s in range(NS):
                    oat = opool.tile([P, ST, OM], f32, tag="o", name=f"oar{s}")
                    nc.sync.dma_start(oat[:], sup(oa_d, s))
                    oa_res.append(oat)
                    oft = opool.tile([P, ST, OM], f32, tag="o", name=f"ofr{s}")
                    nc.scalar.dma_start(oft[:], sup(of_d, s))
                    of_res.append(oft)
                    for j in range(ST):
                        k = s * ST + j
                        for name, ot in (("a", oat), ("f", oft)):
                            trp = ptr.tile([P, OM], f32, tag="tr",
                                           name=f"tr{k}{name}")
                            for i in range(4):
                                nc.tensor.transpose(
                                    trp[:, i * P:(i + 1) * P],
                                    ot[:, j, i * P:(i + 1) * P], ident[:])
                            ob = tpool.tile([P, OM], bf16, tag="obT",
                                            name=f"obT{k}{name}")
                            nc.vector.tensor_copy(ob[:], trp[:])
                            obT[(k, name)] = ob

            # ---- phase G: gather matmuls + per-row reductions ----
            with tc.tile_pool(name="pg", bufs=4, space="PSUM") as pgat:
                for s in range(NS):
                    vt = spool.tile([P, ST, M], f32, tag="v")
                    eng = nc.sync if s % 2 == 0 else nc.scalar
                    eng.dma_start(vt[:], sup(v_d, s))
                    for j in range(ST):
                        k = s * ST + j
                        gps = {}
                        for name, p_sb in (("a", pa_sb), ("f", pf_sb)):
                            ob = obT[(k, name)]
                            gp = pgat.tile([P, M], f32, tag="g",
                                           name=f"gp{k}{name}")
                            for i in range(4):
                                for nh in range(2):
                                    nc.tensor.matmul(
                                        gp[:, nh * 512:(nh + 1) * 512],
                                        lhsT=ob[:, i * P:(i + 1) * P],
                                        rhs=p_sb[:, i, nh * 512:(nh + 1) * 512],
                                        start=(i == 0),
                                        stop=(i == 3),
                                    )
                            gps[name] = gp
                        # stage only the A-gather in SBUF (DVE reads at most
                        # one PSUM operand per op)
                        apg = gpool.tile([P, M], f32, tag="gsb",
                                         name=f"gsb{k}")
                        nc.vector.tensor_copy(apg[:], gps["a"][:])
                        fpg = gps["f"]
                        vtj = vt[:, j, :]
                        # batched products -> two multi-slot reductions
                        prodA = qpool.tile([P, 3, M], f32, tag="prodA",
                                           name=f"prodA{k}")
                        nc.vector.tensor_mul(prodA[:, 0, :], vtj, apg[:])
                        nc.vector.tensor_mul(prodA[:, 1, :], vtj, fpg[:])
                        nc.vector.tensor_mul(prodA[:, 2, :], apg[:], fpg[:])
                        prodB = qpool.tile([P, 4, 512], f32, tag="prodB",
                                           name=f"prodB{k}")
                        nc.vector.tensor_mul(
                            prodB[:, 0:2, :].rearrange("p a b -> p (a b)"),
                            vtj, vtj)
                        nc.vector.tensor_mul(prodB[:, 2, :],
                                             oa_res[s][:, j, :],
                                             oa_res[s][:, j, :])
                        nc.vector.tensor_mul(prodB[:, 3, :],
                                             of_res[s][:, j, :],
                                             of_res[s][:, j, :])
                        st = stat_all[:, k, :]
                        nc.vector.reduce_sum(st[:, 0:3], prodA[:], axis=X)
                        nc.vector.reduce_sum(st[:, 3:7], prodB[:], axis=X)

            nc.sync.dma_start(
                st_d.rearrange("(k p) c -> p k c", p=P), stat_all[:])
    nc.finalize()
    return nc


# ---------------------------------------------------------------- host math
def _greedy_ext(sim):
    om, m = sim.shape
    used = np.zeros(m, dtype=bool)
    I = np.empty(om, dtype=np.int32)
    for r in range(om):
        row = np.where(used, -np.inf, sim[r])
        c = int(np.argmax(row))
        I[r] = c
        used[c] = True
    ext = np.empty(m, dtype=np.int32)
    ext[:om] = I
    ext[om:] = np.nonzero(~used)[0]
    return ext


def _triplet_weights(label, seq_len, vaf_avf):
    f32 = np.float32
    y = np.asarray(label).astype(np.int64)
    n_idx = np.nonzero(y == 0)[0]
    a_idx = np.nonzero(y == 1)[0]
    W = np.zeros((B, T, 4), f32)
    ar = np.arange(T)
    Nn, Na = len(n_idx), len(a_idx)
    if Nn and Na:
        for b in n_idx:
            L = int(seq_len[b])
            W[b, :, 0] = (ar < L).astype(f32) / (f32(L) * Nn)
        for b in a_idx:
            L = int(seq_len[b])
            k = L // 16 + 1
            sig = np.asarray(vaf_avf[b], np.float64)
            valid = ar < L
            o_s = np.argsort(np.where(valid, sig, np.inf), kind="stable")
            o_l = np.argsort(np.where(valid, -sig, np.inf), kind="stable")
            W[b, o_s[:k], 1] = 1.0 / (f32(k) * Na)
            W[b, o_l[:k], 2] = 1.0 / (f32(k) * Na)
    return W, Nn, Na


_runner_cache = {}


def _make_runner(nc):
    """Cached variant of bass2jax.run_bass_via_pjrt's multi-core path: jit
    once per program, reuse the compiled executable across kernel() calls."""
    import jax
    import numpy as _np
    from jax.experimental.shard_map import shard_map
    from jax.sharding import Mesh, PartitionSpec
    from concourse import bass2jax, mybir

    bass2jax.install_neuronx_cc_hook()
    assert nc.dbg_addr is None or not nc.dbg_callbacks
    partition_name = (nc.partition_id_tensor.name
                      if nc.partition_id_tensor else None)
    in_names, out_names, out_avals, zero_shapes = [], [], [], []
    for alloc in nc.m.functions[0].allocations:
        if not isinstance(alloc, mybir.MemoryLocationSet):
            continue
        name = alloc.memorylocations[0].name
        if alloc.kind == "ExternalInput":
            if name != partition_name:
                in_names.append(name)
        elif alloc.kind == "ExternalOutput":
            shape = tuple(alloc.tensor_shape)
            dtype = mybir.dt.np(alloc.dtype)
            out_names.append(name)
            out_avals.append(jax.core.ShapedArray(shape, dtype))
            zero_shapes.append((shape, dtype))
    n_params = len(in_names)
    n_outs = len(out_names)
    all_in = list(in_names) + list(out_names)
    if partition_name is not None:
        all_in.append(partition_name)
    donate = tuple(range(n_params, n_params + n_outs))

    def _body(*args):
        operands = list(args)
        if partition_name is not None:
            operands.append(bass2jax.partition_id_tensor())
        return tuple(bass2jax._bass_exec_p.bind(
            *operands,
            out_avals=tuple(out_avals),
            in_names=tuple(all_in),
            out_names=tuple(out_names),
            lowering_input_output_aliases=(),
            sim_require_finite=True,
            sim_require_nnan=True,
            nc=nc,
        ))

    devices = jax.devices()[:N_CORES]
    mesh = Mesh(_np.asarray(devices), ("core",))
    in_specs = (PartitionSpec("core"),) * (n_params + n_outs)
    out_specs = (PartitionSpec("core"),) * n_outs
    sharded = jax.jit(
        shard_map(_body, mesh=mesh, in_specs=in_specs, out_specs=out_specs,
                  check_rep=False),
        donate_argnums=donate, keep_unused=True)

    def run(in_maps):
        concat_in = [
            np.concatenate([np.asarray(m[name]) for m in in_maps], axis=0)
            for name in in_names
        ]
        concat_zeros = [
            np.zeros((N_CORES * s[0], *s[1:]), d) for (s, d) in zero_shapes
        ]
        out_arrs = sharded(*concat_in, *concat_zeros)
        return [
            {name: np.asarray(out_arrs[i]).reshape(
                N_CORES, *out_avals[i].shape)[c]
             for i, name in enumerate(out_names)}
            for c in range(N_CORES)
        ]

    return run


def _run_spmd(nc, in_maps):
    key = id(nc)
    if key not in _runner_cache:
        _runner_cache[key] = _make_runner(nc)
    return _runner_cache[key](in_maps)


def kernel(v_satt, va_satt, vf_satt, vaf_satt, v_avf, va_avf, vf_avf, vaf_avf,
           va_out, vf_out, vaf_out, lamda1, lamda2, lamda3, lamda4,
           label, seq_len):
    f32 = np.float32
    v = np.ascontiguousarray(np.asarray(v_satt, f32))
    oa = np.ascontiguousarray(np.asarray(va_satt, f32))
    of = np.ascontiguousarray(np.asarray(vf_satt, f32))
    vaf = np.ascontiguousarray(np.asarray(vaf_satt, f32))

    W, Nn, Na = _triplet_weights(label, seq_len, vaf_avf)

    if "p1" not in _prog_cache:
        import os
        _prog_cache["p1"] = _build_pass1(
            RPC, g_f32r=os.environ.get("G_F32R", "1") == "1")
    if "p2" not in _prog_cache:
        _prog_cache["p2"] = _build_pass2(RPC)

    def core_slice(x, c):
        return np.ascontiguousarray(
            x[c * SPC:(c + 1) * SPC].reshape(RPC, -1))

    in1 = [
        dict(v=core_slice(v, c), oa=core_slice(oa, c), of=core_slice(of, c),
             vaf=core_slice(vaf, c), w=core_slice(W, c))
        for c in range(N_CORES)
    ]
    res1 = _run_spmd(_prog_cache["p1"], in1)

    G_A = np.zeros((OM, M), np.float64)
    G_F = np.zeros((OM, M), np.float64)
    csq = np.zeros((3, M), np.float64)
    Tm = np.zeros((4, M), np.float64)
    for r in res1:
        G_A += r["ga"]
        G_F += r["gf"]
        csq += r["csq"]
        Tm += r["tm"]
    csqV = csq[0]
    csqA = csq[1, :OM]
    csqF = csq[1, OM:]

    nV = np.maximum(np.sqrt(csqV), 1e-12)
    simA = G_A / np.maximum(np.sqrt(csqA), 1e-12)[:, None] / nV[None, :]
    simF = G_F / np.maximum(np.sqrt(csqF), 1e-12)[:, None] / nV[None, :]
    extA = _greedy_ext(simA.astype(f32))
    extF = _greedy_ext(simF.astype(f32))

    def one_hot(ext):
        Pm = np.zeros((OM, M), ml_dtypes.bfloat16)
        j = np.arange(M)
        sel = ext < OM
        Pm[ext[sel], j[sel]] = 1.0
        return Pm

    in2 = [
        dict(v=core_slice(v, c), oa=core_slice(oa, c), of=core_slice(of, c),
             pa=one_hot(extA), pf=one_hot(extF))
        for c in range(N_CORES)
    ]
    res2 = _run_spmd(_prog_cache["p2"], in2)
    stats = np.concatenate([r["st"] for r in res2], axis=0)  # [B*T, 8]

    n1, n2, n3 = (stats[:, i].astype(np.float64) for i in range(3))
    rnV = np.sqrt(stats[:, 3].astype(np.float64) + stats[:, 4].astype(np.float64))
    rnA = np.sqrt(stats[:, 5].astype(np.float64))
    rnF = np.sqrt(stats[:, 6].astype(np.float64))

    def cos_term(num, rx, ry):
        den = np.maximum(rx * ry, 1e-8)
        return (1.0 - num / den).reshape(B, T).mean(1).sum()

    d_sum = (cos_term(n1, rnV, rnA) + cos_term(n2, rnV, rnF)
             + cos_term(n3, rnA, rnF)) / B

    ar = np.arange(T)
    seqm = (ar[None, :] < np.asarray(seq_len)[:, None]).astype(np.float64)
    Vs = np.asarray(v_avf, np.float64) * seqm
    As = np.asarray(va_avf, np.float64) * seqm
    Fs = np.asarray(vf_avf, np.float64) * seqm

    def ce(q, p):
        e = 1e-6
        q = np.clip(q, e, 1 - e)
        p = np.clip(p, e, 1 - e)
        return -(p * np.log(q) + (1 - p) * np.log(1 - q)).mean()

    ma_loss = d_sum + ce(Vs, As) + ce(Vs, Fs) + ce(As, Fs)

    yf = np.asarray(label).astype(np.float64)

    def bce(p, yy):
        p = np.asarray(p, np.float64)
        return -(yy * np.log(p) + (1 - yy) * np.log(1 - p)).mean()

    a_loss = bce(va_out, yf)
    f_loss = bce(vf_out, yf)
    raf_loss = bce(vaf_out, yf)

    if Nn == 0 or Na == 0:
        trip = 0.0
    else:
        anchor, pos, neg = Tm[0], Tm[1], Tm[2]
        nrm = lambda x: x / np.linalg.norm(x)
        a_, p_, g_ = nrm(anchor), nrm(pos), nrm(neg)
        d = lambda x, z: np.linalg.norm(x - z + 1e-6)
        trip = max(d(a_, p_) - d(a_, g_) + 5.0, 0.0)

    lam = [float(lamda1), float(lamda2), float(lamda3), float(lamda4)]
    total = (lam[0] * ma_loss + lam[1] * (a_loss + f_loss)
             + lam[2] * raf_loss + lam[3] * trip)
    return np.array([total, ma_loss, a_loss + f_loss, raf_loss, trip], f32)

